# revision 19
# baseline (speedup 1.0000x reference)
"""CrossCompressUnit TRN2 kernel.

v_out = v * (e.w_vv) + e * (v.w_ev) + bias_v
e_out = v * (e.w_ve) + e * (v.w_ee) + bias_e

Data-parallel over batch across 8 NeuronCores (2048 rows/core).
Host interleaves e/v per 128-row block so each supertile moves with ONE
2MB DMA each way on the Sync HWDGE ring (one-time loads ride the Scalar
HWDGE ring so they never delay the input stream).

Per 128-row block ("granule"):
  - four per-row dot products, each ONE fused VectorE scalar_tensor_tensor:
    out=(src*1.0)*w_k with accum_out = the dot
  - v_out: diagonal matmuls on TensorE accumulating in PSUM
    (diag = identity scaled per-partition on ScalarE), evacuated by ScalarE
  - e_out: t4 = e*s_ee on ScalarE (activation scale), then ONE fused
    VectorE scalar_tensor_tensor: (v*s_ve) + t4 -> SBUF directly
"""

import numpy as np
from contextlib import ExitStack

import concourse.bass as bass
import concourse.bacc as bacc
import concourse.tile as tile
from concourse import mybir
from concourse import bass_utils

NCORES = 8
B = 16384
D = 1024
RPC = B // NCORES          # rows per core
P = 128                    # partitions
NBLK = RPC // P            # 16 row-blocks per core
NPG = 2                    # row-blocks per supertile (2MB stacked DMAs)
NST = NBLK // NPG          # supertiles per core

F32 = mybir.dt.float32

_built = {}
LAST_RESULT = None
TRACE = False


def _build(with_bias: bool):
    nc = bacc.Bacc(
        "TRN2",
        target_bir_lowering=False,
        debug=False,
        enable_asserts=False,
        num_devices=NCORES,
    )

    # host interleaves per row-block: ve[n, 0] = e rows, ve[n, 1] = v rows
    ve_d = nc.dram_tensor("ve", [NBLK, 2, P, D], F32, kind="ExternalInput").ap()
    w_d = nc.dram_tensor("wcat", [1, 4 * D], F32, kind="ExternalInput").ap()
    id_d = nc.dram_tensor("ident", [P, P], F32, kind="ExternalInput").ap()
    if with_bias:
        b_d = nc.dram_tensor("bcat", [1, 2 * D], F32, kind="ExternalInput").ap()
    # out[n, 0] = v_out rows, out[n, 1] = e_out rows
    o_d = nc.dram_tensor("veout", [NBLK, 2, P, D], F32, kind="ExternalOutput").ap()

    # [128, 16, 2, 1024]: partition = row within block, n = row-block, s = e/v
    ver = ve_d.rearrange("n s p d -> p n s d")
    our = o_d.rearrange("n s p d -> p n s d")

    MULT = mybir.AluOpType.mult
    ADD = mybir.AluOpType.add
    COPY = mybir.ActivationFunctionType.Copy

    with tile.TileContext(nc) as tc:
        with ExitStack() as ctx:
            singles = ctx.enter_context(tc.tile_pool(name="singles", bufs=1))
            io_pool = ctx.enter_context(tc.tile_pool(name="io", bufs=4))
            t_pool = ctx.enter_context(tc.tile_pool(name="t", bufs=3))
            dg_pool = ctx.enter_context(tc.tile_pool(name="diag", bufs=3))
            sm_pool = ctx.enter_context(tc.tile_pool(name="small", bufs=6))
            ps_pool = ctx.enter_context(
                tc.tile_pool(name="psum", bufs=2, space="PSUM")
            )

            # one-time setup. Weights arrive as a single tiny [1, 4096] row
            # (16KB DMA on the Scalar ring) and are broadcast across the 128
            # partitions on-device with K=1 ones-matmuls on the idle PE --
            # this keeps the Sync ring + HBM stream free for ve/out traffic.
            # order: w_vv, w_ev, w_ve, w_ee multiplied against (e, v, e, v)
            wrow = singles.tile([1, 4 * D], F32)
            nc.scalar.dma_start(out=wrow, in_=w_d)
            ones1 = singles.tile([1, P], F32)
            nc.vector.memset(ones1, 1.0)
            ident = singles.tile([P, P], F32)
            nc.gpsimd.dma_start(out=ident, in_=id_d)

            wbs = []
            H = 512
            for k in range(4):
                wbk = singles.tile([P, D], F32, name=f"wb{k}")
                wps = ps_pool.tile([P, D], F32, tag="wps", bufs=1)
                for h in range(D // H):
                    nc.tensor.matmul(
                        wps[:, h * H : (h + 1) * H],
                        ones1,
                        wrow[0:1, k * D + h * H : k * D + (h + 1) * H],
                        start=True, stop=True,
                    )
                nc.scalar.copy(out=wbk, in_=wps)
                wbs.append(wbk)

            if with_bias:
                brow = singles.tile([1, 2 * D], F32)
                nc.scalar.dma_start(out=brow, in_=b_d)
                beb = singles.tile([P, D], F32)
                b_bcast = bass.AP(
                    tensor=b_d.tensor,
                    offset=b_d.offset + D,
                    ap=[[0, P], [1, D]],
                )
                nc.gpsimd.dma_start(out=beb, in_=b_bcast)

            chunks = [(0, 1), (1, 1)]
            b0 = 2
            while b0 < NBLK:
                chunks.append((b0, NPG))
                b0 += NPG
            for t0, npg in chunks:
                blk = slice(t0, t0 + npg)
                nb = 2 if npg == 1 else 3
                ve = io_pool.tile([P, npg, 2, D], F32, tag=f"ve{npg}", bufs=nb)
                nc.sync.dma_start(out=ve, in_=ver[:, blk, :, :])
                ou = io_pool.tile([P, npg, 2, D], F32, tag=f"ou{npg}", bufs=nb)

                for g in range(npg):
                    eg = ve[:, g, 0, :]
                    vg = ve[:, g, 1, :]

                    # dots: s0 = e.w_vv, s1 = v.w_ev, s2 = e.w_ve, s3 = v.w_ee
                    # each is ONE fused DVE op: out=(src*1)*w_k, accum=dot
                    s = sm_pool.tile([P, 4], F32, tag="dots")
                    garbage = t_pool.tile([P, D], F32, tag="garbage")
                    for k, src in enumerate((eg, vg, eg, vg)):
                        nc.vector.scalar_tensor_tensor(
                            out=garbage,
                            in0=src,
                            scalar=1.0,
                            in1=wbs[k],
                            op0=MULT,
                            op1=MULT,
                            accum_out=s[:, k : k + 1],
                        )

                    # v_out = s0*v + s1*e via PE diag matmuls
                    dgs = dg_pool.tile([P, 2, P], F32, tag="dg")
                    nc.scalar.activation(
                        out=dgs[:, 0, :], in_=ident, func=COPY, scale=s[:, 0:1]
                    )
                    nc.scalar.activation(
                        out=dgs[:, 1, :], in_=ident, func=COPY, scale=s[:, 1:2]
                    )

                    vps = ps_pool.tile([P, D], F32, tag="vps")
                    H = 512
                    for h in range(D // H):
                        sl = slice(h * H, (h + 1) * H)
                        nc.tensor.matmul(
                            vps[:, sl], dgs[:, 0, :], vg[:, sl],
                            start=True, stop=False,
                        )
                        nc.tensor.matmul(
                            vps[:, sl], dgs[:, 1, :], eg[:, sl],
                            start=False, stop=not with_bias,
                        )
                        if with_bias:
                            nc.tensor.matmul(
                                vps[:, sl], ones1, brow[0:1, sl],
                                start=False, stop=True,
                            )
                    nc.scalar.copy(out=ou[:, g, 0, :], in_=vps)

                    # e_out = s2*v + s3*e: t4 on ScalarE, fused mix-add on DVE
                    t4 = t_pool.tile([P, D], F32, tag="t4")
                    nc.scalar.activation(
                        out=t4, in_=eg, func=COPY, scale=s[:, 3:4]
                    )
                    nc.vector.scalar_tensor_tensor(
                        out=ou[:, g, 1, :],
                        in0=vg,
                        scalar=s[:, 2:3],
                        in1=t4,
                        op0=MULT,
                        op1=ADD,
                    )
                    if with_bias:
                        nc.vector.tensor_tensor(
                            out=ou[:, g, 1, :],
                            in0=ou[:, g, 1, :],
                            in1=beb,
                            op=ADD,
                        )

                    nc.sync.dma_start(
                        out=our[:, t0 + g : t0 + g + 1, :, :],
                        in_=ou[:, g : g + 1, :, :],
                    )

    nc.compile()
    return nc


def _get(with_bias: bool):
    if with_bias not in _built:
        _built[with_bias] = _build(with_bias)
    return _built[with_bias]


def kernel(v, e, weight_vv, weight_ev, weight_ve, weight_ee, bias_v, bias_e):
    global LAST_RESULT
    v = np.asarray(v, dtype=np.float32)
    e = np.asarray(e, dtype=np.float32)
    bias_v = np.asarray(bias_v, dtype=np.float32)
    bias_e = np.asarray(bias_e, dtype=np.float32)
    with_bias = bool(np.any(bias_v) or np.any(bias_e))

    nc = _get(with_bias)

    wcat = np.concatenate(
        [
            np.asarray(w, dtype=np.float32).reshape(-1)
            for w in (weight_vv, weight_ev, weight_ve, weight_ee)
        ]
    )
    wcat = wcat.reshape(1, 4 * D)
    ident = np.eye(P, dtype=np.float32)
    bcat = np.concatenate([bias_v.reshape(-1), bias_e.reshape(-1)]).reshape(1, -1)

    # interleave per 128-row block: [NBLK_total, 2, P, D], s=0 e, s=1 v
    ve = np.ascontiguousarray(
        np.stack([e.reshape(-1, P, D), v.reshape(-1, P, D)], axis=1)
    )

    in_maps = []
    for c in range(NCORES):
        blocks = slice(c * NBLK, (c + 1) * NBLK)
        m = {"ve": ve[blocks], "wcat": wcat, "ident": ident}
        if with_bias:
            m["bcat"] = bcat
        in_maps.append(m)

    res = bass_utils.run_bass_kernel_spmd(
        nc, in_maps, core_ids=list(range(NCORES)), trace=TRACE
    )
    LAST_RESULT = res

    out = np.concatenate([r["veout"] for r in res.results], axis=0)  # [NBLK*8,2,P,D]
    vout = out[:, 0].reshape(B, D)
    eout = out[:, 1].reshape(B, D)
    return (vout, eout)
